# revision 4
# baseline (speedup 1.0000x reference)
"""Tensor-parallel Llama attention (GQA) on 8 TRN2 NeuronCores.

Strategy:
  - Head-sharded QKV + attention: core m computes Q heads [4m, 4m+4) and
    KV head m (GQA group is exactly per-core, so no KV duplication).
  - All matmuls run in bf16 with f32 PSUM accumulation.
  - Everything is kept in transposed [feature, seq] layout so the PE
    contraction dim (partition) is always natural; RoPE's rotate_half is
    applied with a small permutation matmul (R @ qT) instead of
    cross-partition copies.
  - Softmax without max-subtraction (scores for this problem are O(17),
    far below f32 exp overflow); row sums come for free from an
    appended ones-column on V.
  - AllToAll converts head-sharding to sequence-sharding, then each core
    runs o_proj for its 256 rows against the full (transposed) Wo.
  - Host gathers by concatenating the 8 [S/8, HID] outputs.
"""

import numpy as np
import ml_dtypes

H, KV, D, HID = 32, 8, 128, 4096
NCORES = 8
HPC = H // NCORES          # q heads per core
ROWS_Q = HPC * D           # q projection rows per core
P = 128
QCHUNK = 512               # attention q-chunk (score matmul free dim)
QS = 512                   # qkv-phase seq chunk
ROPE_THETA = 10000.0
BF = ml_dtypes.bfloat16


def _patch_tile_drain():
    """This container's walrus build rejects a Drain instruction carrying
    semaphore waits ("Too many sync wait commands"). Re-emit the Tile tail
    drain's waits as standalone single-wait SP instructions, which the
    same walrus accepts, followed by a wait-free drain."""
    from concourse.tile import TileContext
    from concourse.vector_clock import ScopedClock

    if getattr(TileContext, "_drain_waits_patched", False):
        return

    def _drain_and_barrier(self, tick_clock, wait_clock):
        nc = self.nc
        probe = nc.sync.drain()
        wait_clock.add_sem_waits(
            probe.ins, ScopedClock({None: tick_clock.global_clock})
        )
        waits = list(probe.ins.sync_info.on_wait)
        probe.ins.sync_info.on_wait = []
        id2handle = {h.num: h for h in self.sems.allocated().values()}
        for w in waits:
            assert w.wait_mode == "sem-ge-imm", w
            h = id2handle.get(w.id)
            if h is not None:
                nc.sync.wait_ge(h, w.wait_value)
        nc.all_engine_barrier()
        popped = nc._tile_sem_poison_stack.pop()
        assert popped is self._sem_poison
        nc.clear_and_free_semaphores(list(self.sems.allocated().values()))
        nc.all_engine_barrier()

    TileContext._drain_and_barrier = _drain_and_barrier
    TileContext._drain_waits_patched = True

    # This walrus also rejects >1 sync wait on ordinary instructions.
    # Rewrite the BIR before compile: hoist excess waits onto standalone
    # single-wait EventSemaphore instructions on the same engine, placed
    # immediately before the owning instruction (same program order).
    import json as _json

    import concourse.bass2jax as _b2j
    import concourse.bass_utils as _bu

    def _split_bir_multiwaits(bir_json):
        j = _json.loads(bir_json)
        for f in j["functions"]:
            for bb in f["blocks"]:
                out = []
                for ins in bb["instructions"]:
                    si = ins.get("sync_info")
                    ow = (si or {}).get("on_wait") or []
                    if len(ow) > 1:
                        keep, hoist = [], []
                        for w in ow:
                            if w.get("wait_mode") == "sem-ge-imm":
                                hoist.append(w)
                            else:
                                keep.append(w)
                        if not keep and hoist:
                            keep.append(hoist.pop())
                        if len(keep) > 1:
                            raise RuntimeError(
                                f"can't split waits on {ins['name']}: {keep}"
                            )
                        for i, w in enumerate(hoist):
                            out.append(
                                {
                                    "debug": ins.get("debug", 0),
                                    "engine": ins["engine"],
                                    "ins": [],
                                    "outs": [],
                                    "name": f"{ins['name']}.hw{i}",
                                    "opcode": "EventSemaphore",
                                    "sync_info": {
                                        "on_update": [],
                                        "on_wait": [w],
                                    },
                                }
                            )
                        si["on_wait"] = keep
                    out.append(ins)
                bb["instructions"] = out
        return _json.dumps(j).encode()

    _orig_cbk = _bu.compile_bir_kernel

    def _cbk(bir_json, tmpdir, neff_name="file.neff"):
        return _orig_cbk(_split_bir_multiwaits(bir_json), tmpdir, neff_name)

    _bu.compile_bir_kernel = _cbk
    _b2j.compile_bir_kernel = _cbk


def build_nc(S):
    from contextlib import ExitStack

    import concourse.bass as bass
    import concourse.mybir as mybir
    from concourse.tile import TileContext

    _patch_tile_drain()

    f32 = mybir.dt.float32
    bf = mybir.dt.bfloat16

    CHUNK = S // NCORES    # output rows per core
    NST = CHUNK // P       # seq tiles per core in o_proj
    NHC = HID // P         # hidden chunks
    NKT = S // P           # key tiles
    NQC = S // QCHUNK      # attention q chunks
    NQTR = S // QS         # qkv-phase seq chunks
    HH = HID // 2          # o_proj half width

    nc = bass.Bass(num_devices=NCORES)
    xT = nc.declare_dram_parameter("xT", [HID, S], bf, isOutput=False)
    wq = nc.declare_dram_parameter("wq", [HID, ROWS_Q], bf, isOutput=False)
    wk = nc.declare_dram_parameter("wk", [HID, D], bf, isOutput=False)
    wv = nc.declare_dram_parameter("wv", [HID, D], bf, isOutput=False)
    wo = nc.declare_dram_parameter("wo", [HID, HID], bf, isOutput=False)
    cosq = nc.declare_dram_parameter("cosq", [D, S], f32, isOutput=False)
    sinq = nc.declare_dram_parameter("sinq", [D, S], f32, isOutput=False)
    cosk = nc.declare_dram_parameter("cosk", [D, S], f32, isOutput=False)
    sink = nc.declare_dram_parameter("sink", [D, S], f32, isOutput=False)
    rT = nc.declare_dram_parameter("rT", [D, D], f32, isOutput=False)
    maskT = nc.declare_dram_parameter("maskT", [P, 896], bf, isOutput=False)
    ident = nc.declare_dram_parameter("ident", [P, P], bf, isOutput=False)
    out = nc.declare_dram_parameter("out", [CHUNK, HID], f32, isOutput=True)

    a2a_send = nc.dram_tensor("a2a_send", [NCORES, ROWS_Q, CHUNK], bf)
    a2a_recv = nc.dram_tensor("a2a_recv", [NCORES, ROWS_Q, CHUNK], bf)

    with TileContext(nc, num_cores=NCORES) as tc, ExitStack() as top:
        consts = top.enter_context(tc.tile_pool(name="consts", bufs=1))
        persist = top.enter_context(tc.tile_pool(name="persist", bufs=1))

        cosq_sb = consts.tile([D, S], f32, name="cosq_sb")
        nc.sync.dma_start(out=cosq_sb, in_=cosq[:, :])
        sinq_sb = consts.tile([D, S], f32, name="sinq_sb")
        nc.sync.dma_start(out=sinq_sb, in_=sinq[:, :])
        cosk_sb = consts.tile([D, S], f32, name="cosk_sb")
        nc.sync.dma_start(out=cosk_sb, in_=cosk[:, :])
        sink_sb = consts.tile([D, S], f32, name="sink_sb")
        nc.sync.dma_start(out=sink_sb, in_=sink[:, :])
        rT_sb = consts.tile([D, D], f32, name="rT_sb")
        nc.sync.dma_start(out=rT_sb, in_=rT[:, :])
        maskT_sb = consts.tile([P, 896], bf, name="maskT_sb")
        nc.sync.dma_start(out=maskT_sb, in_=maskT[:, :])
        ident_sb = consts.tile([P, P], bf, name="ident_sb")
        nc.sync.dma_start(out=ident_sb, in_=ident[:, :])

        qT_sb = [persist.tile([D, S], bf, name=f"qT{h}") for h in range(HPC)]
        kT_sb = persist.tile([D, S], bf, name="kT_sb")
        vT_sb = persist.tile([D, S], bf, name="vT_sb")
        vnat = persist.tile([P, NKT, D + 1], bf, name="vnat")
        nc.vector.memset(vnat[:, :, D : D + 1], 1.0)

        # ---- phase 1: qkv projections + rope ----
        with ExitStack() as ph1:
            w_pool = ph1.enter_context(tc.tile_pool(name="w_pool", bufs=1))
            xq_pool = ph1.enter_context(tc.tile_pool(name="xq_pool", bufs=2))
            pre_pool = ph1.enter_context(tc.tile_pool(name="pre_pool", bufs=2))
            tmp_pool = ph1.enter_context(tc.tile_pool(name="tmp_pool", bufs=2))
            acc_pool = ph1.enter_context(
                tc.tile_pool(name="acc_pool", bufs=2, space="PSUM")
            )
            rot_pool = ph1.enter_context(
                tc.tile_pool(name="rot_pool", bufs=2, space="PSUM")
            )
            vtr_pool = ph1.enter_context(
                tc.tile_pool(name="vtr_pool", bufs=2, space="PSUM")
            )

            wq_all = w_pool.tile([P, NHC, ROWS_Q], bf, name="wq_all")
            nc.sync.dma_start(
                out=wq_all, in_=wq.ap().rearrange("(a p) c -> p a c", p=P)
            )
            wk_all = w_pool.tile([P, NHC, D], bf, name="wk_all")
            nc.sync.dma_start(
                out=wk_all, in_=wk.ap().rearrange("(a p) c -> p a c", p=P)
            )
            wv_all = w_pool.tile([P, NHC, D], bf, name="wv_all")
            nc.sync.dma_start(
                out=wv_all, in_=wv.ap().rearrange("(a p) c -> p a c", p=P)
            )

            xT_r = xT.ap().rearrange("(a p) s -> p a s", p=P)

            for qtr in range(NQTR):
                sl = slice(qtr * QS, (qtr + 1) * QS)
                xq = xq_pool.tile([P, NHC, QS], bf, tag="xq", name=f"xq{qtr}")
                nc.sync.dma_start(out=xq, in_=xT_r[:, :, sl])

                jobs = [("k", 0), ("v", 0)] + [("q", h) for h in range(HPC)]
                for kind, h in jobs:
                    acc = acc_pool.tile(
                        [P, QS], f32, tag="acc", name=f"acc_{qtr}_{kind}{h}"
                    )
                    for hc in range(NHC):
                        if kind == "q":
                            lhsT = wq_all[:, hc, h * D : (h + 1) * D]
                        elif kind == "k":
                            lhsT = wk_all[:, hc, :]
                        else:
                            lhsT = wv_all[:, hc, :]
                        nc.tensor.matmul(
                            acc,
                            lhsT=lhsT,
                            rhs=xq[:, hc, :],
                            start=(hc == 0),
                            stop=(hc == NHC - 1),
                        )
                    if kind == "v":
                        nc.scalar.copy(out=vT_sb[:, sl], in_=acc)
                        continue
                    pre = pre_pool.tile(
                        [P, QS], f32, tag="pre", name=f"pre_{qtr}_{kind}{h}"
                    )
                    nc.scalar.copy(out=pre, in_=acc)
                    rotp = rot_pool.tile(
                        [P, QS], f32, tag="rot", name=f"rot_{qtr}_{kind}{h}"
                    )
                    nc.tensor.matmul(rotp, lhsT=rT_sb, rhs=pre, start=True, stop=True)
                    if kind == "q":
                        cos_t, sin_t, dest = cosq_sb, sinq_sb, qT_sb[h]
                    else:
                        cos_t, sin_t, dest = cosk_sb, sink_sb, kT_sb
                    tcos = tmp_pool.tile(
                        [P, QS], f32, tag="tcos", name=f"tcos_{qtr}_{kind}{h}"
                    )
                    nc.vector.tensor_mul(tcos, pre, cos_t[:, sl])
                    trot = tmp_pool.tile(
                        [P, QS], f32, tag="trot", name=f"trot_{qtr}_{kind}{h}"
                    )
                    nc.vector.tensor_mul(trot, rotp, sin_t[:, sl])
                    nc.vector.tensor_add(dest[:, sl], tcos, trot)

                for t in range(QS // P):
                    kt = qtr * (QS // P) + t
                    vtr = vtr_pool.tile([P, P], bf, tag="vtr", name=f"vtr{kt}")
                    nc.tensor.transpose(
                        vtr, vT_sb[:, kt * P : (kt + 1) * P], ident_sb
                    )
                    nc.scalar.copy(out=vnat[:, kt, 0:D], in_=vtr)

        # ---- phase 2: attention (S_T layout, no-max softmax) ----
        with ExitStack() as ph2:
            pt_pool = ph2.enter_context(tc.tile_pool(name="pt_pool", bufs=4))
            ob_pool = ph2.enter_context(tc.tile_pool(name="ob_pool", bufs=2))
            ot_pool = ph2.enter_context(tc.tile_pool(name="ot_pool", bufs=2))
            r_pool = ph2.enter_context(tc.tile_pool(name="r_pool", bufs=2))
            sp_pool = ph2.enter_context(
                tc.tile_pool(name="sp_pool", bufs=3, space="PSUM")
            )
            outp_pool = ph2.enter_context(
                tc.tile_pool(name="outp_pool", bufs=1, space="PSUM")
            )
            trp_pool = ph2.enter_context(
                tc.tile_pool(name="trp_pool", bufs=1, space="PSUM")
            )

            for h in range(HPC):
                for qc in range(NQC):
                    nkt = (qc + 1) * (QCHUNK // P)
                    outps = [
                        outp_pool.tile(
                            [P, 512], f32, tag=f"outp{j}", name=f"outp_{h}_{qc}_{j}"
                        )
                        for j in range(4)
                    ]
                    q_sl = slice(qc * QCHUNK, (qc + 1) * QCHUNK)
                    for kt in range(nkt):
                        sp = sp_pool.tile(
                            [P, QCHUNK], f32, tag="sp", name=f"sp_{h}_{qc}_{kt}"
                        )
                        nc.tensor.matmul(
                            sp,
                            lhsT=kT_sb[:, kt * P : (kt + 1) * P],
                            rhs=qT_sb[h][:, q_sl],
                            start=True,
                            stop=True,
                        )
                        pt = pt_pool.tile(
                            [P, QCHUNK], bf, tag="pt", name=f"pt_{h}_{qc}_{kt}"
                        )
                        nc.scalar.activation(
                            pt, sp, mybir.ActivationFunctionType.Exp
                        )
                        j = kt - (nkt - 4)
                        if j >= 0:
                            nc.vector.tensor_mul(
                                pt, pt, maskT_sb[:, 384 - 128 * j : 896 - 128 * j]
                            )
                        for j4 in range(4):
                            nc.tensor.matmul(
                                outps[j4][:, 0 : D + 1],
                                lhsT=pt[:, j4 * P : (j4 + 1) * P],
                                rhs=vnat[:, kt, :],
                                start=(kt == 0),
                                stop=(kt == nkt - 1),
                            )
                    for j4 in range(4):
                        qt = qc * 4 + j4
                        r = r_pool.tile([P, 1], f32, tag="r", name=f"r_{h}_{qt}")
                        nc.vector.reciprocal(r, outps[j4][:, D : D + 1])
                        ob = ob_pool.tile([P, D], bf, tag="ob", name=f"ob_{h}_{qt}")
                        nc.vector.tensor_scalar_mul(ob, outps[j4][:, 0:D], r)
                        trp = trp_pool.tile(
                            [P, P], bf, tag="trp", name=f"trp_{h}_{qt}"
                        )
                        nc.tensor.transpose(trp, ob, ident_sb)
                        ot = ot_pool.tile([P, P], bf, tag="ot", name=f"ot_{h}_{qt}")
                        nc.scalar.copy(out=ot, in_=trp)
                        core_j, col = divmod(qt, NST)
                        nc.sync.dma_start(
                            out=a2a_send[
                                core_j, h * P : (h + 1) * P, col * P : (col + 1) * P
                            ],
                            in_=ot,
                        )

        nc.gpsimd.collective_compute(
            "AllToAll",
            mybir.AluOpType.bypass,
            replica_groups=[list(range(NCORES))],
            ins=[a2a_send[:, :, :]],
            outs=[a2a_recv[:, :, :]],
        )

        # ---- phase 3: o_proj on this core's sequence chunk ----
        with ExitStack() as ph3:
            att_pool = ph3.enter_context(tc.tile_pool(name="att_pool", bufs=1))
            wo_pool = ph3.enter_context(tc.tile_pool(name="wo_pool", bufs=3))
            osb_pool = ph3.enter_context(tc.tile_pool(name="osb_pool", bufs=2))
            o_psum = ph3.enter_context(
                tc.tile_pool(name="o_psum", bufs=1, space="PSUM")
            )

            att_sb = att_pool.tile([P, NHC, CHUNK], bf, name="att_sb")
            nc.sync.dma_start(
                out=att_sb,
                in_=a2a_recv.ap().rearrange("c (b p) s -> p (c b) s", p=P),
            )

            for half in range(2):
                h_sl = slice(half * HH, (half + 1) * HH)
                pos = [
                    o_psum.tile(
                        [P, HH], f32, tag=f"po{st}", name=f"po_{half}_{st}"
                    )
                    for st in range(NST)
                ]
                for fc in range(NHC):
                    wo_sb = wo_pool.tile(
                        [P, HH], bf, tag="wo_sb", name=f"wo_{half}_{fc}"
                    )
                    nc.sync.dma_start(
                        out=wo_sb, in_=wo[fc * P : (fc + 1) * P, h_sl]
                    )
                    for st in range(NST):
                        for s4 in range(HH // 512):
                            nc.tensor.matmul(
                                pos[st][:, s4 * 512 : (s4 + 1) * 512],
                                lhsT=att_sb[:, fc, st * P : (st + 1) * P],
                                rhs=wo_sb[:, s4 * 512 : (s4 + 1) * 512],
                                start=(fc == 0),
                                stop=(fc == NHC - 1),
                            )
                for st in range(NST):
                    osb = osb_pool.tile(
                        [P, HH], f32, tag="osb", name=f"osb_{half}_{st}"
                    )
                    nc.scalar.copy(out=osb, in_=pos[st])
                    nc.sync.dma_start(
                        out=out[st * P : (st + 1) * P, h_sl], in_=osb
                    )

    return nc


def make_in_maps(x, Wq, Wk, Wv, Wo):
    S = x.shape[1]
    xT = np.ascontiguousarray(x.reshape(S, HID).T.astype(np.float32)).astype(BF)
    woT = np.ascontiguousarray(Wo.astype(np.float32).T).astype(BF)

    inv_freq = 1.0 / (
        ROPE_THETA ** (np.arange(0, D, 2, dtype=np.float32) / np.float32(D))
    )
    t = np.arange(S, dtype=np.float32)
    freqs = np.outer(t, inv_freq).astype(np.float32)
    emb = np.concatenate([freqs, freqs], axis=1)
    cosT = np.cos(emb).T.astype(np.float32)  # [D, S]
    sinT = np.sin(emb).T.astype(np.float32)
    scale = np.float32(1.0 / np.sqrt(np.float32(D)))
    cosq = np.ascontiguousarray(cosT * scale)
    sinq = np.ascontiguousarray(sinT * scale)
    cosk = np.ascontiguousarray(cosT)
    sink = np.ascontiguousarray(sinT)

    R = np.zeros((D, D), dtype=np.float32)
    for i in range(D // 2):
        R[i, i + D // 2] = -1.0
        R[i + D // 2, i] = 1.0
    rT = np.ascontiguousarray(R.T)

    mask = np.zeros((P, 896), dtype=np.float32)
    for k in range(P):
        mask[k, k + 384 :] = 1.0
    maskT = mask.astype(BF)
    ident = np.eye(P, dtype=np.float32).astype(BF)

    in_maps = []
    for m in range(NCORES):
        wqT = np.ascontiguousarray(
            Wq[m * ROWS_Q : (m + 1) * ROWS_Q, :].astype(np.float32).T
        ).astype(BF)
        wkT = np.ascontiguousarray(
            Wk[m * D : (m + 1) * D, :].astype(np.float32).T
        ).astype(BF)
        wvT = np.ascontiguousarray(
            Wv[m * D : (m + 1) * D, :].astype(np.float32).T
        ).astype(BF)
        in_maps.append(
            dict(
                xT=xT,
                wq=wqT,
                wk=wkT,
                wv=wvT,
                wo=woT,
                cosq=cosq,
                sinq=sinq,
                cosk=cosk,
                sink=sink,
                rT=rT,
                maskT=maskT,
                ident=ident,
            )
        )
    return in_maps


def gather_out(results, S):
    parts = [np.asarray(results[c]["out"], dtype=np.float32) for c in range(NCORES)]
    return np.concatenate(parts, axis=0).reshape(1, S, HID)


def kernel(x, Wq, Wk, Wv, Wo):
    from concourse.bass_utils import run_bass_kernel_spmd

    x = np.asarray(x)
    S = x.shape[1]
    nc = build_nc(S)
    in_maps = make_in_maps(x, np.asarray(Wq), np.asarray(Wk), np.asarray(Wv), np.asarray(Wo))
    res = run_bass_kernel_spmd(nc, in_maps, list(range(NCORES)))
    return gather_out(res.results, S)
